# revision 1
# baseline (speedup 1.0000x reference)
"""MiniBatchDiscrimination kernel for 8 Trainium2 NeuronCores.

Problem:
  x [256, 1024] f32, T [1024, 128, 16] f32
  M = einsum('na,abc->nbc', x, T)                      [N=256, B=128, C=16]
  D[k,j,b] = sum_c |M[k,b,c] - M[j,b,c]|
  Cmat = exp(-D); S = sum_j Cmat
  out = S - Cmat[:, N-1, :]; out[0] = S[0]-Cmat[0,0]; out[N-1] = S[N-1]-Cmat[N-1,N-1]

Sharding: data-parallel over B (each core owns 16 of the 128 b-channels).
The pairwise distance is independent per b, so there is no communication.

Per-core dataflow (abs decomposed as |d| = 2*relu(d) - d, since abs_max is
not a valid TRN2 DVE ALU op but relu (sub,max,0) is a single 2x-mode op;
the linear term sum_c d = R[b,j] - R[b,k] is folded in by PE and the exp
bias). Everything streams in fp16 except the f32 PSUM accumulations:
  PE   : MT[bc, n] = (x @ T_loc)^T via 16 accumulating matmuls (a-chunks)
  PE   : R[b, j] = sum_c M[j, b, c]  (pattern matmul)
  DVE  : per k: relu(MT[:, j] - MT[:, k]) via tensor_scalar(sub, max, 0),
         4x perf mode; 1 of 8 k's runs on ScalarE activation(Relu) instead,
         emitted one group ahead so ScalarE's FIFO can't stall PE
  PE   : c-reduction: 2*pattern^T @ relu-tile -> 2P, 4 k's per [128, 256]
         PSUM bank via col-group tile_position (16-row slices at 32g); one
         fold matmul per bank adds -R[b, j]
  ScE  : exp(-psD + bias), bias = -R[b,k] per partition; accum_out emits
         the row sums S (the j-reduction) for free
  Pool : extract Cmat[:, 255] columns (and Cmat[0,0])
  DVE  : final out = S - C255 (+ k=0 self fix), in two halves to overlap
         the output DMAs with the second half of the main loop
"""

import os
import sys

import numpy as np

for _p in ("/opt/trn_rl_repo", os.path.expanduser("~/.axon_site/_ro/trn_rl_repo")):
    if os.path.isdir(_p) and _p not in sys.path:
        sys.path.insert(0, _p)
        break

import concourse.bass as bass
import concourse.tile as tile
from concourse import bacc, mybir
from concourse.bass_utils import run_bass_kernel_spmd

A, B, C, N = 1024, 128, 16, 256
NCORES = 8
BL = B // NCORES          # 16 b-channels per core
BC = BL * C               # 256 (b, c) pairs per core
NGROUPS = N // 8          # 32 groups of 8 k-values
F32 = mybir.dt.float32
ALU = mybir.AluOpType
AF = mybir.ActivationFunctionType

DT_STREAM = mybir.dt.float16  # dtype of the absdiff stream path (F32 or float16)
F32R = mybir.dt.float32r


def on_act(k: int) -> bool:
    """k's whose relu-diff runs on ScalarE (load balancing vs DVE)."""
    return k % 8 == 3 and k % 32 != 27


_cache = {}


def _patterns():
    # patA maps bc-block0 partitions (b = p//16 in 0..7) to out row b;
    # patB maps bc-block1 partitions to out rows 8 + p//16.
    patA = np.zeros((128, 16), np.float32)
    patB = np.zeros((128, 16), np.float32)
    for p in range(128):
        patA[p, p // 16] = 1.0
        patB[p, 8 + p // 16] = 1.0
    # fold weight: out[32g + b, :] += rhs[b, :]
    foldW = np.zeros((16, 128), np.float32)
    for m in range(128):
        if m % 32 < 16:
            foldW[m % 32, m] = 1.0
    return patA, patB, foldW


def build_program(dbg: bool = False):
    nc = bacc.Bacc(
        "TRN2", target_bir_lowering=False, debug=False, enable_asserts=True
    )

    xT_d = nc.dram_tensor("xT", [A, N], DT_STREAM, kind="ExternalInput")
    tl_d = nc.dram_tensor("Tl", [A, BC], DT_STREAM, kind="ExternalInput")
    out_d = nc.dram_tensor("out", [N, BL], F32, kind="ExternalOutput")
    if dbg:
        mt_o = nc.dram_tensor("mt_o", [2, 128, N], F32, kind="ExternalOutput")
        negR_o = nc.dram_tensor("negR_o", [16, N], F32, kind="ExternalOutput")
        psD_o = nc.dram_tensor("psD_o", [2, 128, 512], F32, kind="ExternalOutput")
        sall_o = nc.dram_tensor("sall_o", [128, 64], F32, kind="ExternalOutput")
        c255_o = nc.dram_tensor("c255_o", [128, 64], F32, kind="ExternalOutput")

    np_dt = np.float32 if DT_STREAM == F32 else np.float16
    patA_np, patB_np, foldW_np = _patterns()
    pats_np = np.concatenate(
        [patA_np, patB_np, 2 * patA_np, 2 * patB_np], axis=1)  # [128, 64]
    pats_d = nc.inline_tensor(pats_np.astype(np_dt), name="pats")
    foldW_d = nc.inline_tensor(foldW_np.astype(np_dt), name="foldW")

    xT_v = xT_d[:].rearrange("(a p) n -> p a n", p=128)
    tl_v = tl_d[:].rearrange("(a p) m -> p a m", p=128)

    with tile.TileContext(nc) as tc:
        with (
            tc.tile_pool(name="persist", bufs=1) as pp,
            tc.tile_pool(name="ad", bufs=32) as adp,
            tc.tile_pool(name="cm", bufs=8) as cmp_,
            tc.tile_pool(name="psum_d", bufs=6, space="PSUM") as pd,
        ):
            # ---- PE clock warmup: the HAM gate holds PE at half clock
            # until ~3.5us of sustained activity; PE would idle during the
            # input DMAs anyway, so burn that window on dummy matmuls and
            # run the real GEMM at full clock ----
            warm_t = pp.tile([128, 128], DT_STREAM, tag="warm")
            nc.vector.memset(warm_t[:], 0.0)
            pwm_ctx = tc.tile_pool(name="psum_warm", bufs=1, space="PSUM")
            pwm = pwm_ctx.__enter__()
            wps = pwm.tile([128, 128], F32, tag="wps")
            for _ in range(32):
                nc.tensor.matmul(wps[:], warm_t[:], warm_t[:],
                                 start=True, stop=True)
            pwm_ctx.__exit__(None, None, None)

            # ---- load inputs (split across both HWDGE rings: SP and ACT) ----
            xbig = pp.tile([128, 8 * N], DT_STREAM, tag="xbig")
            tbig = pp.tile([128, 8 * BC], DT_STREAM, tag="tbig")
            xbv = xbig[:].rearrange("p (a n) -> p a n", a=8)
            tbv = tbig[:].rearrange("p (a m) -> p a m", a=8)
            # first a-chunk alone so the GEMM can start ~1.5us in
            nc.sync.dma_start(xbv[:, 0:1], xT_v[:, 0:1])
            nc.scalar.dma_start(tbv[:, 0:1], tl_v[:, 0:1])
            nc.sync.dma_start(xbv[:, 1:4], xT_v[:, 1:4])
            nc.scalar.dma_start(tbv[:, 1:4], tl_v[:, 1:4])
            nc.scalar.dma_start(xbv[:, 4:8], xT_v[:, 4:8])
            nc.sync.dma_start(tbv[:, 4:8], tl_v[:, 4:8])
            xts = [xbig[:, a * N:(a + 1) * N] for a in range(8)]
            tls = [tbig[:, a * BC:(a + 1) * BC] for a in range(8)]

            pats_t = pp.tile([128, 64], DT_STREAM, tag="pats")
            nc.sync.dma_start(pats_t[:], pats_d[:])
            pats = {
                "patA1": pats_t[:, 0:16], "patB1": pats_t[:, 16:32],
                "patA2": pats_t[:, 32:48], "patB2": pats_t[:, 48:64],
            }
            foldW_t = pp.tile([16, 128], DT_STREAM, tag="foldW")
            nc.scalar.dma_start(foldW_t[:], foldW_d[:])

            # ---- GEMM: MT[bc, n] = sum_a Tl[a, bc] * x[n, a] ----
            pmt_ctx = tc.tile_pool(name="psum_mt", bufs=2, space="PSUM")
            pmt = pmt_ctx.__enter__()
            MT = []        # stream dtype (input of absdiff)
            MTs = []       # f32 scalar source for tensor_scalar scalar1
            negMT = []     # f32, bias source for ScalarE Abs
            for blk in range(2):
                ps = pmt.tile([128, N], F32, tag="psmt")
                for a in range(8):
                    nc.tensor.matmul(
                        ps[:],
                        tls[a][:, blk * 128:(blk + 1) * 128],
                        xts[a],
                        start=(a == 0),
                        stop=(a == 7),
                    )
                mt_t = pp.tile([128, N], DT_STREAM, tag=f"mt{blk}")
                nc.scalar.copy(mt_t[:], ps[:])
                if DT_STREAM == F32:
                    mts_t = mt_t
                else:
                    mts_t = pp.tile([128, N], F32, tag=f"mts{blk}")
                    nc.vector.tensor_copy(mts_t[:], mt_t[:])
                nmt_t = pp.tile([128, N], F32, tag=f"nmt{blk}")
                nc.vector.tensor_scalar(
                    out=nmt_t[:], in0=mts_t[:], scalar1=-1.0, scalar2=None,
                    op0=ALU.mult,
                )
                MT.append(mt_t)
                MTs.append(mts_t)
                negMT.append(nmt_t)

            # ---- R[b, j] = sum_c M[j, b, c]; negR = -R ----
            psR = pmt.tile([16, N], F32, tag="psmt")
            nc.tensor.matmul(psR[:], pats["patA1"], MT[0][:],
                             start=True, stop=False)
            nc.tensor.matmul(psR[:], pats["patB1"], MT[1][:],
                             start=False, stop=True)
            # negR in stream dtype: the fold matmul adds exactly these values,
            # and the exp bias below must cancel them bit-exactly on j == k.
            negR = pp.tile([16, N], DT_STREAM, tag="negR")
            nc.scalar.mul(negR[:], psR[:], -1.0)
            pmt_ctx.__exit__(None, None, None)

            # negRbias: [128, 64]; col 2G+h rows 32g+b = -R[b, 8G+4h+g]
            negRb = pp.tile([128, 2 * NGROUPS], DT_STREAM, tag="negRb")
            nc.vector.memset(negRb[:], 0.0)
            for g in range(4):
                src = negR[:].rearrange("b (q g) -> b q g", g=4)[:, :, g]
                nc.sync.dma_start(negRb[32 * g:32 * g + 16, :], src)

            # ---- persistent result tiles ----
            S_all = pp.tile([128, 2 * NGROUPS], F32, tag="S_all")
            C255 = pp.tile([128, 2 * NGROUPS], F32, tag="C255")
            C00 = pp.tile([16, 1], F32, tag="C00")
            R_all = pp.tile([128, 2 * NGROUPS], F32, tag="R_all")
            # out row k = 8G+4h+g, col b  <-  R_all[32g+b, 2G+h]
            dstv = out_d[:].rearrange("(G h g) b -> g b G h", G=NGROUPS, h=2, g=4)

            def make_act_ads(GG, store):
                for h in range(2):
                    for g in range(4):
                        k = 8 * GG + 4 * h + g
                        if not on_act(k):
                            continue
                        pair = []
                        for blk in range(2):
                            ad_t = adp.tile([128, N], DT_STREAM, tag="adact")
                            nc.scalar.activation(
                                ad_t[:], MT[blk][:], AF.Relu,
                                bias=negMT[blk][:, k:k + 1], scale=1.0,
                            )
                            pair.append(ad_t)
                        store[k] = pair

            def finalize_half(lo, hi):
                # out[k] = S[k] - Cmat[k, 255] for cols [lo, hi)
                nc.vector.tensor_tensor(
                    out=R_all[:, lo:hi], in0=S_all[:, lo:hi],
                    in1=C255[:, lo:hi], op=ALU.subtract,
                )
                if lo == 0:
                    # out[0] = S[0] - Cmat[0, 0]
                    nc.vector.tensor_tensor(
                        out=R_all[0:16, 0:1], in0=S_all[0:16, 0:1],
                        in1=C00[:], op=ALU.subtract,
                    )
                # (k=255 -> col 63: its C255 value IS Cmat[255,255]; no fix)
                for g in range(4):
                    srcv = R_all[32 * g:32 * g + 16, lo:hi].rearrange(
                        "b (G h) -> b G h", h=2)
                    nc.sync.dma_start(dstv[g][:, lo // 2:hi // 2, :], srcv)

            # ---- main loop ----
            act_ads = {}
            make_act_ads(0, act_ads)
            for G in range(NGROUPS):
                if G + 1 < NGROUPS:
                    # ScalarE relu-diffs for the NEXT group, queued ahead of
                    # this group's exps so a blocked exp can't delay them
                    make_act_ads(G + 1, act_ads)
                if G == NGROUPS // 2:
                    finalize_half(0, NGROUPS)
                for h in range(2):
                    # own PSUM bank per half: the exp (ACT read) must not
                    # share a bank with the next half's PE writes, or Tile
                    # serializes them (bank-overlap tracking).
                    psDh = pd.tile([128, N], F32, tag="psD")
                    for g in range(4):
                        k = 8 * G + 4 * h + g
                        if on_act(k):
                            ads = act_ads.pop(k)
                        else:
                            ads = []
                            for blk in range(2):
                                ad_t = adp.tile([128, N], DT_STREAM, tag="ad")
                                nc.vector.tensor_scalar(
                                    out=ad_t[:], in0=MT[blk][:],
                                    scalar1=MTs[blk][:, k:k + 1], scalar2=0.0,
                                    op0=ALU.subtract, op1=ALU.max,
                                )
                                ads.append(ad_t)
                        outsl = psDh[32 * g:32 * g + 16, :]
                        nc.tensor.matmul(
                            outsl, pats["patA2"], ads[0][:],
                            start=True, stop=False, tile_position=(0, 32 * g),
                        )
                        nc.tensor.matmul(
                            outsl, pats["patB2"], ads[1][:],
                            start=False, stop=False,
                            tile_position=(0, 32 * g),
                        )
                    # psDh += -R[b, j] broadcast over the four 32-row groups
                    nc.tensor.matmul(
                        psDh[:], foldW_t[:], negR[:],
                        start=False, stop=True, skip_group_check=True,
                    )
                    if dbg and G in (0, 4):
                        dcp = cmp_.tile([128, N], F32, tag="dcp")
                        nc.vector.tensor_copy(dcp[:], psDh[:])
                        nc.sync.dma_start(
                            psD_o[:][0 if G == 0 else 1][:, N * h:N * (h + 1)],
                            dcp[:])
                    col = 2 * G + h
                    cm_t = cmp_.tile([128, N], F32, tag="cm")
                    nc.scalar.activation(
                        cm_t[:], psDh[:], AF.Exp,
                        bias=negRb[:, col:col + 1],
                        scale=-1.0,
                        accum_out=S_all[:, col:col + 1],
                    )
                    nc.gpsimd.tensor_copy(C255[:, col:col + 1], cm_t[:, 255:256])
                    if G == 0 and h == 0:
                        # Cmat[0, 0, :] lives at rows 0..15, j-col 0 (k=0 is g=0)
                        nc.gpsimd.tensor_copy(C00[:], cm_t[0:16, 0:1])

            finalize_half(NGROUPS, 2 * NGROUPS)

            if dbg:
                for blk in range(2):
                    nc.sync.dma_start(mt_o[:][blk], MTs[blk][:])
                nc.sync.dma_start(negR_o[:], negR[:])
                nc.sync.dma_start(sall_o[:], S_all[:])
                nc.sync.dma_start(c255_o[:], C255[:])

    nc.compile()
    return nc


def kernel(x: np.ndarray, T: np.ndarray) -> np.ndarray:
    if "nc" not in _cache:
        _cache["nc"] = build_program()
    nc = _cache["nc"]

    np_dt = np.float32 if DT_STREAM == F32 else np.float16
    x = np.ascontiguousarray(x, dtype=np.float32)
    T = np.ascontiguousarray(T, dtype=np.float32)
    xT = np.ascontiguousarray(x.T.astype(np_dt))         # [A, N]

    in_maps = []
    for c in range(NCORES):
        tl = np.ascontiguousarray(
            T[:, c * BL:(c + 1) * BL, :].reshape(A, BC).astype(np_dt))
        in_maps.append({"xT": xT, "Tl": tl})

    res = run_bass_kernel_spmd(nc, in_maps, list(range(NCORES)))
    outs = [res.results[c]["out"] for c in range(NCORES)]
    return np.concatenate(outs, axis=1)                  # [N, B]


if __name__ == "__main__":
    rng = np.random.default_rng(0)
    x = rng.standard_normal((N, A)).astype(np.float32)
    T = rng.random((A, B, C), dtype=np.float32)
    out = kernel(x, T)
    print(out.shape, out.dtype, out[:3, :3])



# revision 3
# speedup vs baseline: 1.1151x; 1.1151x over previous
"""MiniBatchDiscrimination kernel for 8 Trainium2 NeuronCores — v2.

Problem:
  x [256, 1024] f32, T [1024, 128, 16] f32
  M = einsum('na,abc->nbc', x, T)                      [N=256, B=128, C=16]
  D[k,j,b] = sum_c |M[k,b,c] - M[j,b,c]|
  Cmat = exp(-D); S = sum_j Cmat
  out = S - Cmat[:, N-1, :]; out[0] = S[0]-Cmat[0,0]; out[N-1] = S[N-1]-Cmat[N-1,N-1]

Sharding: data-parallel over B (each core owns 16 of the 128 b-channels);
the pairwise distance is independent per b, no communication.

v2 exploits the symmetry D[k,j] = D[j,k]: only the strict upper triangle
(j > k) is computed.  out[k] = rowsum_k + colsum_k + 1 - C[k,255] with the
self term added exactly as the constant 1 (tiny fixups for k in {0,255}).

Each group G packs its 8 k's into one [128, W] PSUM bank at 16-row slots
(rows 16g+b, k = 8G+g, tile col t = j - 8G - 1, W = 255 - 8G): one exp /
rowsum-accum / colsum / C255-extract per 8 k's.  16-row output placement
uses full-partition [128, 128] shifted-pattern lhsT (zeros elsewhere
accumulate harmlessly) instead of 32-aligned tile_position.

Per-core dataflow (stream dtype f16 except f32 PSUM accumulation):
  PE   : MT[bc, n] = (x @ T_loc)^T via 16 accumulating matmuls
  PE   : R[b, j] = sum_c M[j, b, c]  (pattern matmul)
  elementwise pairwise tiles, two complementary forms:
    k < TS (relu form, DVE tensor_scalar, 4x mode, one op per (k, block)):
         ad = relu(M[:, j] - M[:, k]),  D = 2*sum_c ad - R_j + R_k
    k >= TS (max form, diagonal bands, DVE/Pool tensor_tensor, 2x mode):
         AX[bc, d, k] = max(MT[bc, k+d], MT[bc, k]) for a [16-delta x Kc]
         rectangle per chunk — one instruction covers many k at once;
         D = 2*sum_c AX - R_j - R_k.  PE reads per-k delta-slices of the
         chunk via strided access patterns.
  PE   : c-reduction into the psD bank; a full-width fold matmul
         (start=True) seeds -R_j; one stair matmul poisons the j <= k
         leading corner so exp -> 0; one colW matmul per group
         accumulates column sums (the j < k half of S) into a
         persistent [16, 256] PSUM bank
  ScE  : exp(-psD + bias), bias = -+R[b,k] (f32, built by PE matmuls)
         per partition; accum_out emits row sums into S_all
  Pool : extracts Cmat[:, 255] columns; produces the early low-k band
         chunks and a few relu-form tiles (DVE is the busier engine)
  DVE  : finalize out = S_all + colsum + 1 - C255 (+fixups), 1 out DMA
         (host unshuffles the [16g+b, G] layout)
"""

import os
import sys

import numpy as np

for _p in ("/opt/trn_rl_repo", os.path.expanduser("~/.axon_site/_ro/trn_rl_repo")):
    if os.path.isdir(_p) and _p not in sys.path:
        sys.path.insert(0, _p)
        break

import concourse.bass as bass
import concourse.tile as tile
from concourse import bacc, mybir
from concourse.bass_utils import run_bass_kernel_spmd

A, B, C, N = 1024, 128, 16, 256
NCORES = 8
BL = B // NCORES          # 16 b-channels per core
BC = BL * C               # 256 (b, c) pairs per core
NGROUPS = N // 8          # 32 groups of 8 k-values
F32 = mybir.dt.float32
F16 = mybir.dt.float16
ALU = mybir.AluOpType
AF = mybir.ActivationFunctionType

TS = 88                   # k < TS: DVE per-k relu form
TS_P = 104                # k in [TS, TS_P): Pool per-k relu form (groups 11-12)
TSG = TS_P // 8           # groups below this are relu form (bias -R)
DB = 16                   # delta band height
KC = 64                   # max k-extent of a band chunk
BIG = 32768.0             # leading-corner poison value (exp -> 0)

# band d covers deltas [1+DB*d, 1+DB*(d+1)) for k in [TS_P, 255-DB*d)
NBANDS = (N - 1 - TS_P + DB - 1) // DB

# constants tile column layout (f16, [128, CW])
C_PATA1 = 0        # [128, 16] patA (1.0)
C_PATB1 = 16       # [128, 16] patB (1.0)
C_COLW = 32        # [128, 16] colsum weights
C_FOLD = 64        # rows 0:16, [16, 128] fold weights
C_IDW = 192        # rows 0:16, [16, 256] shifted id16 (slide for 16g offs)
C_STAIR = 448      # [128, 7] leading-poison stair (BIG)
C_LEADW = 456      # [128, 128] leading-poison row selector
C_PAW_A = 584      # [128, 240] shifted patA (2.0), slide for 16g offsets
C_PAW_B = 824      # [128, 240] shifted patB (2.0)
CW = 1064


def _bands():
    bands = []
    for d in range(NBANDS):
        dlo = 1 + DB * d
        kd = (N - 1) - dlo - TS_P + 1        # number of valid k (k from TS_P)
        chunks = []
        k0 = TS_P
        while k0 < TS_P + kd:
            kc = min(KC, TS_P + kd - k0)
            chunks.append((k0, kc))
            k0 += kc
        bands.append((d, dlo, kd, chunks))
    return bands


# relu-form k's whose tensor_scalar pair runs on Pool instead of DVE
# (the real Pool engine supports TensorScalarPtr but NOT TensorTensor, so
# Pool cannot make diagonal band chunks; it contributes whole relu-form
# groups G11-G12 early plus a sprinkling of k's in the DVE relu region)
POOL_TS = ({k for k in range(TS) if k % 11 == 5}
           | set(range(TS, TS_P)))


def _consts():
    c = np.zeros((128, CW), np.float32)
    for p in range(128):
        c[p, C_PATA1 + p // 16] = 1.0
        c[p, C_PATB1 + 8 + p // 16] = 1.0
        c[p, C_COLW + p % 16] = 1.0
        # leadW[q, p] = 1 iff q == p//16  (lhsT: partition q, out col p)
        c[p // 16, C_LEADW + p] = 1.0
    # stair[q, j] = BIG iff j < q (for q = row-group index 0..7)
    for q in range(8):
        for j in range(q):
            c[q, C_STAIR + j] = BIG
    # foldW[b, m] = 1 iff b == m % 16 (rows 0:16)
    for m in range(128):
        c[m % 16, C_FOLD + m] = 1.0
    # idwide[b, 112 + b] = 1 (rows 0:16); slice [112-16g : 240-16g]
    for b in range(16):
        c[b, C_IDW + 112 + b] = 1.0
    # patAwide[p, 112 + p//16] = 2; patBwide[p, 112 + 8 + p//16] = 2
    for p in range(128):
        c[p, C_PAW_A + 112 + p // 16] = 2.0
        c[p, C_PAW_B + 112 + 8 + p // 16] = 2.0
    return c


def _ovl(ap2d, base, dims):
    """3D view of a 2D [128, *] AP: free dims = dims [[stride, count], ...]
    starting at column `base`; allows overlapping reads."""
    v = ap2d[:, base:base + 1]
    for _ in range(len(dims) - 1):
        v = v.unsqueeze(1)
    v = v.broadcast_to([ap2d.shape[0]] + [d[1] for d in dims])
    c = v.copy()
    for i, d in enumerate(dims):
        c.ap[i + 1] = list(d)
    return c


_cache = {}


def build_program(dbg: bool = False):
    nc = bacc.Bacc(
        "TRN2", target_bir_lowering=False, debug=False, enable_asserts=True
    )

    xT_d = nc.dram_tensor("xT", [A, N], F16, kind="ExternalInput")
    tl_d = nc.dram_tensor("Tl", [A, BC], F16, kind="ExternalInput")
    # raw result layout [16g+b, G] = out[k=8G+g, b]; host unshuffles
    out_d = nc.dram_tensor("out", [128, NGROUPS], F32, kind="ExternalOutput")
    if dbg:
        sall_o = nc.dram_tensor("sall_o", [128, 32], F32, kind="ExternalOutput")
        c255_o = nc.dram_tensor("c255_o", [128, 32], F32, kind="ExternalOutput")
        csum_o = nc.dram_tensor("csum_o", [16, 256], F16, kind="ExternalOutput")

    pats_d = nc.inline_tensor(_consts().astype(np.float16), name="pats")

    xT_v = xT_d[:].rearrange("(a p) n -> p a n", p=128)
    tl_v = tl_d[:].rearrange("(a p) m -> p a m", p=128)

    MTW = N + DB            # padded MT width (diag reads up to 255 + 16)
    bands = _bands()

    with tile.TileContext(nc) as tc:
        with (
            tc.tile_pool(name="persist", bufs=1) as pp,
            tc.tile_pool(name="ad", bufs=64) as adp,
            tc.tile_pool(name="cm", bufs=14) as cmp_,
            tc.tile_pool(name="psum_d", bufs=5, space="PSUM") as pd,
            tc.tile_pool(name="psum_cs", bufs=1, space="PSUM") as pcs,
        ):
            # ---- PE clock warmup (HAM gate holds PE at half clock until
            # ~3.5us of sustained activity; burn the DMA window) ----
            warm_t = pp.tile([128, 128], F16, tag="warm")
            nc.vector.memset(warm_t[:], 0.0)
            pwm_ctx = tc.tile_pool(name="psum_warm", bufs=1, space="PSUM")
            pwm = pwm_ctx.__enter__()
            wps = pwm.tile([128, 128], F32, tag="wps")
            for _ in range(16):
                nc.tensor.matmul(wps[:], warm_t[:], warm_t[:],
                                 start=True, stop=True)
            pwm_ctx.__exit__(None, None, None)

            # ---- load inputs: few big DMAs (each DMA instruction costs
            # ~625ns of serialized HWDGE queue time) ----
            xbig = pp.tile([128, 8 * N], F16, tag="xbig")
            tbig = pp.tile([128, 8 * BC], F16, tag="tbig")
            xbv = xbig[:].rearrange("p (a n) -> p a n", a=8)
            tbv = tbig[:].rearrange("p (a m) -> p a m", a=8)
            nc.sync.dma_start(xbv[:, 0:8], xT_v[:, 0:8])
            nc.scalar.dma_start(tbv[:, 0:1], tl_v[:, 0:1])
            nc.scalar.dma_start(tbv[:, 1:8], tl_v[:, 1:8])
            xts = [xbig[:, a * N:(a + 1) * N] for a in range(8)]
            tls = [tbig[:, a * BC:(a + 1) * BC] for a in range(8)]

            pats_t = pp.tile([128, CW], F16, tag="pats")
            nc.sync.dma_start(pats_t[:], pats_d[:])
            colW_t = pats_t[:, C_COLW:C_COLW + 16]
            foldW_t = pats_t[0:16, C_FOLD:C_FOLD + 128]
            stair_t = pats_t[:, C_STAIR:C_STAIR + 7]
            leadW_t = pats_t[:, C_LEADW:C_LEADW + 128]

            def paw(g, blk):
                base = C_PAW_A if blk == 0 else C_PAW_B
                lo = base + 112 - 16 * g
                return pats_t[:, lo:lo + 128]

            def idw(g):
                lo = C_IDW + 112 - 16 * g
                return pats_t[0:16, lo:lo + 128]

            # ---- GEMM: MT[bc, n] = sum_a Tl[a, bc] * x[n, a] ----
            pmt_ctx = tc.tile_pool(name="psum_mt", bufs=2, space="PSUM")
            pmt = pmt_ctx.__enter__()
            MT = []        # f16 [128, MTW] (cols 256.. zero padded)
            MTs32 = []     # f32 scalar source for the relu-form tensor_scalar
            for blk in range(2):
                ps = pmt.tile([128, N], F32, tag="psmt")
                for a in range(8):
                    nc.tensor.matmul(
                        ps[:],
                        tls[a][:, blk * 128:(blk + 1) * 128],
                        xts[a],
                        start=(a == 0),
                        stop=(a == 7),
                    )
                mt_t = pp.tile([128, MTW], F16, tag=f"mt{blk}")
                nc.scalar.copy(mt_t[:, 0:N], ps[:])
                nc.gpsimd.memset(mt_t[:, N:MTW], 0.0)
                mts_t = pp.tile([128, TS_P], F32, tag=f"mts{blk}")
                nc.vector.tensor_copy(mts_t[:], mt_t[:, 0:TS_P])
                MT.append(mt_t)
                MTs32.append(mts_t)

            # ---- R[b, j] = sum_c M[j, b, c] ----
            psR = pmt.tile([16, N], F32, tag="psmt")
            nc.tensor.matmul(psR[:], pats_t[:, C_PATA1:C_PATA1 + 16],
                             MT[0][:, 0:N], start=True, stop=False)
            nc.tensor.matmul(psR[:], pats_t[:, C_PATB1:C_PATB1 + 16],
                             MT[1][:, 0:N], start=False, stop=True)
            negR16 = pp.tile([16, N], F16, tag="negR16")   # fold rhs
            nc.scalar.mul(negR16[:], psR[:], -1.0)
            # bias tiles in exp layout via shifted-pattern matmuls (no DMA):
            # psRb[16g+b, G] = 2*R[b, k=8G+g]
            psRb = pmt.tile([128, NGROUPS], F32, tag="psmt")
            nc.tensor.matmul(psRb[:], warm_t[:], xts[0][:, 0:NGROUPS],
                             start=True, stop=False, skip_group_check=True)
            for g in range(8):
                for blk in range(2):
                    rhs = MT[blk][:, 0:N].rearrange(
                        "p (G e) -> p e G", e=8)[:, g, :]
                    nc.tensor.matmul(
                        psRb[:], paw(g, blk), rhs,
                        start=False, stop=(g == 7 and blk == 1),
                        skip_group_check=True,
                    )
            negRb = pp.tile([128, NGROUPS], F32, tag="negRb")
            posRb = pp.tile([128, NGROUPS], F32, tag="posRb")
            nc.scalar.mul(negRb[:], psRb[:], -0.5)
            nc.scalar.mul(posRb[:], psRb[:], 0.5)
            pmt_ctx.__exit__(None, None, None)

            # ---- persistent colsum PSUM bank [16, 256] ----
            csum = pcs.tile([16, N], F32, tag="csum", padded_shape=[16, 512])
            nc.tensor.matmul(csum[:], warm_t[:, 0:16], MT[0][:, 0:N],
                             start=True, stop=False, skip_group_check=True)

            # ---- diagonal max-form band tiles.  DVE chunks are emitted up
            # front (DVE streams through them while PE runs the band-region
            # groups); Pool chunks are deferred into the main loop. ----
            # AX[blk, d, ci] layout: [128, DB * kc], element (dd, kk) at
            # dd*kc+kk = max(MT[:, (k0+kk) + (dlo+dd)], MT[:, k0+kk])
            AX = {}
            deferred = {}   # loop position -> [(blk, d, ci)]
            ts_ads = {}
            ts_done = set()

            def emit_chunk(blk, d, ci, eng):
                dlo = bands[d][1]
                k0, kc = bands[d][3][ci]
                ax_t = AX[(blk, d, ci)]
                in0 = _ovl(MT[blk][:], k0 + dlo, [[1, DB], [1, kc]])
                in1 = MT[blk][:, k0:k0 + kc].unsqueeze(1).broadcast_to(
                    [128, DB, kc])
                out3 = ax_t[:].rearrange("p (d k) -> p d k", d=DB)
                eng.tensor_tensor(out=out3, in0=in0, in1=in1, op=ALU.max)

            def make_ts_ads(GG):
                """relu-form tiles for group GG (k < TS), one op per block."""
                if GG in ts_done:
                    return
                ts_done.add(GG)
                for g in range(8):
                    k = 8 * GG + g
                    w = N - 1 - k
                    eng = nc.gpsimd if k in POOL_TS else nc.vector
                    pair = []
                    for blk in range(2):
                        ad_t = adp.tile([128, w], F16, tag="ad")
                        eng.tensor_scalar(
                            out=ad_t[:], in0=MT[blk][:, k + 1:N],
                            scalar1=MTs32[blk][:, k:k + 1], scalar2=0.0,
                            op0=ALU.subtract, op1=ALU.max,
                        )
                        pair.append(ad_t)
                    ts_ads[k] = pair

            # DVE makes group 0's relu tiles first: PE consumes them at
            # loop position 0 while the band chunks stream in behind.
            # Pool starts on its two groups immediately (consumed mid-band).
            make_ts_ads(0)
            make_ts_ads(12)
            make_ts_ads(11)

            # emission order: chunk k-range descending, then band ascending
            # (PE consumes groups G descending, each tile touching every
            # band's slice for its k's), blocks interleaved
            allchunks = sorted(
                ((d, ci) for (d, dlo, kd, chunks) in bands
                 for ci in range(len(chunks))),
                key=lambda dc: (-bands[dc[0]][3][dc[1]][0], dc[0]))
            for (d, ci) in allchunks:
                k0, kc = bands[d][3][ci]
                for blk in range(2):
                    AX[(blk, d, ci)] = pp.tile(
                        [128, DB * kc], F16,
                        name=f"ax{blk}_{d}_{ci}", tag=f"ax{blk}_{d}_{ci}")
                    emit_chunk(blk, d, ci, nc.vector)

            # ---- persistent result tiles ----
            S_all = pp.tile([128, NGROUPS], F32, tag="S_all")
            C255 = pp.tile([128, NGROUPS], F32, tag="C255")
            R_all = pp.tile([128, NGROUPS], F32, tag="R_all")

            # PE processing order: group 0 first (head-gap filler), the
            # band-region groups descending with the two Pool-made groups
            # interleaved to plug production gaps, then the remaining
            # relu-form groups ascending
            order = ([0] + list(range(NGROUPS - 1, 24, -1)) + [12]
                     + list(range(24, 18, -1)) + [11]
                     + list(range(18, 12, -1)) + list(range(1, 11)))

            for pos, G in enumerate(order):
                # emit relu-form DVE ops two groups ahead of consumption
                if pos + 2 < len(order) and order[pos + 2] < TSG:
                    make_ts_ads(order[pos + 2])
                kmin = 8 * G
                W = N - 1 - kmin
                # pad to a full 2KB bank so matmul outputs stay in-bank
                psD = pd.tile([128, W], F32, tag="psD",
                              padded_shape=[128, 512])
                # fold: psD[16g+b, t] = -R[b, kmin+1+t]  (full width, seeds)
                nc.tensor.matmul(
                    psD[:], foldW_t, negR16[:, kmin + 1:N],
                    start=True, stop=False, skip_group_check=True,
                )
                # poison the leading corner (cols t < g in row-group g)
                nc.tensor.matmul(
                    psD[:, 0:min(7, W)], leadW_t, stair_t[:, 0:min(7, W)],
                    start=False, stop=False, skip_group_check=True,
                )
                for g in range(8):
                    k = kmin + g
                    w = N - 1 - k
                    if w <= 0:
                        continue
                    if G < TSG:
                        ads = ts_ads.pop(k)
                        for blk in range(2):
                            nc.tensor.matmul(
                                psD[:, g:W], paw(g, blk), ads[blk][:],
                                start=False, stop=False,
                                skip_group_check=True,
                            )
                    else:
                        for blk in range(2):
                            for (d, dlo, kd, chunks) in bands:
                                cnt = min(DB, N - 1 - k - DB * d)
                                if cnt <= 0:
                                    break
                                for ci, (k0, kc) in enumerate(chunks):
                                    if k0 <= k < k0 + kc:
                                        break
                                ax_t = AX[(blk, d, ci)]
                                rhs = ax_t[:].rearrange(
                                    "p (d k) -> p d k", d=DB)[:, 0:cnt, k - k0]
                                t0 = g + DB * d
                                nc.tensor.matmul(
                                    psD[:, t0:t0 + cnt], paw(g, blk), rhs,
                                    start=False, stop=False,
                                    skip_group_check=True,
                                )
                # close the psD accumulation group before the ACT read
                nc.tensor.matmul(
                    psD[0:16, 0:1], warm_t[:, 0:16], warm_t[:, 0:1],
                    start=False, stop=True, skip_group_check=True,
                )
                cm_t = cmp_.tile([128, W], F16, tag="cm")
                bias = negRb if G < TSG else posRb
                nc.scalar.activation(
                    cm_t[:], psD[:], AF.Exp,
                    bias=bias[:, G:G + 1],
                    scale=-1.0,
                    accum_out=S_all[:, G:G + 1],
                )
                # column sums (the j < k half of S) accumulate in PSUM
                nc.tensor.matmul(
                    csum[:, kmin + 1:N], colW_t, cm_t[:],
                    start=False, stop=(pos == len(order) - 1),
                    skip_group_check=True,
                )
                nc.vector.tensor_copy(C255[:, G:G + 1], cm_t[:, W - 1:W])

            # ---- finalize ----
            csumS = pp.tile([16, N], F16, tag="csumS")
            nc.scalar.copy(csumS[:], csum[:])
            # scatter colsum into exp layout via shifted-id matmuls (no
            # DMA): csRbP[16g+b, G] = csum[b, k=8G+g]
            csRbP = pcs.tile([128, NGROUPS], F32, tag="csum",
                             padded_shape=[128, 512])
            nc.tensor.matmul(csRbP[:], warm_t[:], xts[0][:, 0:NGROUPS],
                             start=True, stop=False, skip_group_check=True)
            for g in range(8):
                rhs = csumS[:, 0:N].rearrange("b (G e) -> b e G", e=8)[:, g, :]
                nc.tensor.matmul(
                    csRbP[:], idw(g), rhs,
                    start=False, stop=(g == 7), skip_group_check=True,
                )
            nc.vector.tensor_tensor(
                out=R_all[:], in0=S_all[:], in1=csRbP[:], op=ALU.add)
            nc.vector.scalar_tensor_tensor(
                out=R_all[:], in0=R_all[:], scalar=1.0, in1=C255[:],
                op0=ALU.add, op1=ALU.subtract)
            # k = 0 (rows 0:16, col 0): out = rowsum + colsum (no +1 - C255)
            nc.vector.tensor_tensor(
                out=R_all[0:16, 0:1], in0=S_all[0:16, 0:1],
                in1=csRbP[0:16, 0:1], op=ALU.add)
            # (k = 255's spurious +1 is subtracted on the host)
            nc.sync.dma_start(out_d[:], R_all[:])

            if dbg:
                nc.sync.dma_start(sall_o[:], S_all[:])
                nc.sync.dma_start(c255_o[:], C255[:])
                nc.sync.dma_start(csum_o[:], csumS[:])

    nc.compile()
    return nc


def kernel(x: np.ndarray, T: np.ndarray) -> np.ndarray:
    if "nc" not in _cache:
        _cache["nc"] = build_program()
    nc = _cache["nc"]

    x = np.ascontiguousarray(x, dtype=np.float32)
    T = np.ascontiguousarray(T, dtype=np.float32)
    xT = np.ascontiguousarray(x.T.astype(np.float16))    # [A, N]

    in_maps = []
    for c in range(NCORES):
        tl = np.ascontiguousarray(
            T[:, c * BL:(c + 1) * BL, :].reshape(A, BC).astype(np.float16))
        in_maps.append({"xT": xT, "Tl": tl})

    res = run_bass_kernel_spmd(nc, in_maps, list(range(NCORES)))
    outs = []
    for c in range(NCORES):
        outs.append(unshuffle(res.results[c]["out"]))
    return np.concatenate(outs, axis=1)                  # [N, B]


def unshuffle(raw: np.ndarray) -> np.ndarray:
    """raw [16g+b, G] f32 -> out [k = 8G+g, b]  [N, BL]"""
    raw = raw.copy()
    raw[112:128, NGROUPS - 1] -= 1.0            # k = 255 self-term fixup
    r = raw.reshape(8, 16, NGROUPS)             # [g, b, G]
    return np.ascontiguousarray(r.transpose(2, 0, 1).reshape(N, 16))


if __name__ == "__main__":
    rng = np.random.default_rng(0)
    x = rng.standard_normal((N, A)).astype(np.float32)
    T = rng.random((A, B, C), dtype=np.float32)
    out = kernel(x, T)
    print(out.shape, out.dtype, out[:3, :3])
